# revision 25
# baseline (speedup 1.0000x reference)
"""Trainium2 Bass kernel for ArcShapeRadiusConfigVisibleNeighDist.

For each pedestrian i (N=8192):
  heading u_i = normalize(pos_i - past_i)
  over all j: dist_ij = |pos_j - pos_i|, visible iff angle(pos_j-pos_i, u_i)
  in [-35deg, 35deg) and j != i. Output = affine(clip(mean visible dist)).

Key reformulation (no atan2 anywhere):
  visible  <=>  rel . u_i > cos(35deg) * dist  <=>  dot/c > dist
  sq and dot/c are K-small matmuls on the TensorEngine with fp16 hi/lo
  split features (K is free on the PE). Both feature sets are stacked
  into one K=18 block (rows 0-9: G1/sq, rows 10-17: G2/dot); the G1/G2
  stationaries zero the other block's rows, so the matmuls are plain
  unmasked K=18 matmuls.

Clock-gate handling (measured): the PE HAM activity monitor does not
count small-K matmuls as "busy", so sustained K=18 work re-throttles
the array to 1.2 GHz ~3.4us after the last full-array matmul (600ns
per 512-col matmul instead of ~375ns warm). Fix: zero-pad K to 128 so
every production matmul is full-array. Real features sit at partitions
96-113 (the 32-row host tensors land at 96-127 with host-zero rows
114-127); partitions 0-95 are zeroed once on device (base-0 memsets -
non-zero-base engine APs are limited to 32 partitions). The stationary
zero rows select the G1/G2 block and null the pad rows, so K=128
matmuls compute exactly the K=18 product while holding 2.4 GHz. A ~4us
burst of warm-up matmuls on scratch PSUM covers the initial ramp
during the input-DMA wait.

Per 128-query x 1024-j chunk (single fused vector pass per element):
  PE:  g1 = sq (+eps) [128,1024], g2 = dot/c [128,1024]
  ACT: dist = sqrt(g1) -> fp16 [128,1024]
  DVE: custom MASKED_SDC: b = select(g2 > dist, dist + ENC_C, 0),
       accum -> A = ENC_C*cnt + s  (one accumulator carries BOTH the
       visible count and the visible-distance sum; per-chunk s < 2^17
       so the host separates them with a floor-divide).
Host epilogue: cnt = floor((A+64)/ENC_C); s = A - ENC_C*cnt per chunk,
  summed over chunks; r = clip(s/max(cnt,1) * k + b, 0.5, 4.0);
  select by indexes.

Sharding: core k owns queries [k*1024, (k+1)*1024), full j set.
"""

import numpy as np

import concourse.bass as bass
import concourse.bacc as bacc
import concourse.mybir as mybir
import concourse.tile as tile
from contextlib import ExitStack
from concourse.bass_utils import run_bass_kernel_spmd
from concourse.dve_uop import DveOpSpec
import concourse.dve_ops as dvo
from concourse.dve_ops import Spec, Src0, Src1, Zero, C1, select, lower, has_src1
from concourse.dve_ops import AluOp as SAluOp

N = 8192
NCORES = 8
Q = N // NCORES            # 1024 queries per core
ITILES = Q // 128          # 8 partition tiles of queries
JCHUNK = 1024
NJC = N // JCHUNK          # 8 j-chunks per i-tile
EPS = 0.005                # sq guard: keeps diag excluded, sqrt input > 0
COS_HALF = float(np.cos(70.0 * np.pi / 180.0 / 2.0))
MIN_R, MAX_R = 0.5, 4.0
MIN_D, MAX_D = 0.2, 5.0
SLOPE = (MAX_R - MIN_R) / (MAX_D - MIN_D)
OFFS = MIN_R - MIN_D * SLOPE
ENC_C = 131072.0           # 2^17: per-chunk s < 1024*dmax ~ 98e3 < 2^17
KF = 18                    # stacked feature rows: 0-9 G1 (sq), 10-17 G2 (dot)
KFP = 32                   # host pads features to 32 rows (engine ops need
                           # 32-aligned partition bases; rows 18-31 are zero)

F32 = mybir.dt.float32
FP16 = mybir.dt.float16
ACTF = mybir.ActivationFunctionType
_F16 = np.float16


def register_masked_sdc():
    """Runtime-register the fused DVE op:
    out = select(in0 > in1, in1 + s1, 0), accum_out = sum(out).
    With s1 = ENC_C the accumulator encodes ENC_C*count + sum(dist) in one
    fp32 lane. The per-NEFF uop table is generated from OPS, so appending
    at runtime is sufficient (no firmware change)."""
    name = "MASKED_SDC_ANT"
    if name in dvo._SUB_OPCODE_FOR_NAME:
        return getattr(dvo, name)

    def _ref(in0, in1, s0, s1, imm2):
        b = np.where(in1.astype(np.float32) > in0,
                     in0.astype(np.float32) + np.float32(s1),
                     0.0).astype(np.float32)
        return b, b.reshape(b.shape[0], -1).sum(axis=-1, keepdims=True)

    # dist rides in0 (SBUF fp16), the PSUM fp32 operand rides in1
    spec = Spec(body=select(Src1 > Src0, Src0 + C1, Zero), accum=SAluOp.ADD,
                reference=_ref)
    row = max(dvo._SUB_OPCODE_FOR_NAME.values()) + 1
    assert row < 0x20
    dvo._SUB_OPCODE_FOR_NAME[name] = row
    op = dvo.DveOp(name, spec, subdim=False, uops_sha={})
    for ver in ("v3", "v4"):
        s = DveOpSpec(name=name, opcode=row, uops=lower(spec, ver=ver),
                      rd1_en=has_src1(spec))
        op.uops_sha[ver] = s.sha(ver)
    dvo.OPS.append(op)
    dvo.CUSTOM_DVE_SPECS[name] = spec
    setattr(dvo, name, op)
    return op


def _split(x):
    """Split f64 array into fp16 hi + fp16 lo (as f64 of exact fp16 values)."""
    h = x.astype(_F16).astype(np.float64)
    l = (x - h).astype(_F16).astype(np.float64)
    return h, l


def _build_graph():
    masked_sdc = register_masked_sdc()
    nc = bacc.Bacc("TRN2", target_bir_lowering=False, debug=False,
                   num_devices=NCORES)
    # One [32, *] feature stack (rows 0-9 G1, 10-17 G2, 18-31 host zeros);
    # G1/G2 separation happens via zero rows in the two stationary blocks.
    # hs carries both stationaries, hj the first two j-chunks.
    jw = 2 * JCHUNK
    # stationaries split so the slices i-tile 0 needs (cols 0:1280 cover
    # its G1 slice at 0:128 and G2 slice at 1024:1152) land first
    HS0 = 1280
    hs0_d = nc.dram_tensor("hs0", [KFP, HS0], FP16, kind="ExternalInput")
    hsr_d = nc.dram_tensor("hsr", [KFP, 2 * Q - HS0], FP16,
                           kind="ExternalInput")
    hj_d = nc.dram_tensor("hj", [KFP, jw], FP16, kind="ExternalInput")
    t0_d = nc.dram_tensor("t0", [KFP, 2 * JCHUNK], FP16, kind="ExternalInput")
    t1_d = nc.dram_tensor("t1", [KFP, 2 * JCHUNK], FP16, kind="ExternalInput")
    t2_d = nc.dram_tensor("t2", [KFP, 2 * JCHUNK], FP16, kind="ExternalInput")
    oa_d = nc.dram_tensor("out_a", [128, ITILES * NJC], F32,
                          kind="ExternalOutput")

    with tile.TileContext(nc) as tc, ExitStack() as ctx:
        singles = ctx.enter_context(tc.tile_pool(name="singles", bufs=1))
        psum = ctx.enter_context(tc.tile_pool(name="psum", bufs=2, space="PSUM"))
        work = ctx.enter_context(tc.tile_pool(name="work", bufs=4))

        # warm-up operands: zeroed via the (idle) Vector queue so the
        # initial warm-up matmuls can start during the input-DMA wait
        wu_l = singles.tile([128, 128], FP16)
        wu_r = singles.tile([128, 512], FP16)
        nc.gpsimd.memset(wu_l[:], 0.0)
        nc.gpsimd.memset(wu_r[:], 0.0)

        # All matmul operands are [128, *]: rows 0-17 carry the real
        # features (DMA), rows 18-127 are zeroed on device. Full-array
        # K=128 matmuls keep the PE activity monitor happy so the array
        # holds its 2.4 GHz clock (small-K matmuls don't count as "busy"
        # and the PE re-throttles to 1.2 GHz ~3.4us after the last
        # full-array matmul - measured as 600ns vs 215ns per matmul).
        hs = singles.tile([128, 2 * Q], FP16)
        hj = singles.tile([128, jw], FP16)
        t0 = singles.tile([128, 2 * JCHUNK], FP16)
        t1 = singles.tile([128, 2 * JCHUNK], FP16)
        t2 = singles.tile([128, 2 * JCHUNK], FP16)
        # real features live at partitions 96-113 (zeros 114-127 come from
        # the host pad); partitions 0-95 are zeroed on device. Engine APs
        # with non-zero partition base are limited to 32 partitions, so the
        # big memsets must be the base-0 ones. Queue placement orders the
        # chunk-0 dependencies (hs, hj) first on every queue.
        FB = 128 - KFP  # feature base partition (96)
        # sync carries the chunk-0 critical tensors; gpsimd's first dma
        # pays a one-time SWDGE load, so it only gets later tensors
        nc.sync.dma_start(hs[FB:128, 0:HS0], hs0_d[:])
        nc.sync.dma_start(hj[FB:128, :], hj_d[:])
        nc.sync.dma_start(hs[FB:128, HS0:], hsr_d[:])
        nc.sync.dma_start(t1[FB:128, :], t1_d[:])
        nc.gpsimd.dma_start(t0[FB:128, :], t0_d[:])
        nc.gpsimd.dma_start(t2[FB:128, :], t2_d[:])
        # pad-row zeroing, chunk-0 pieces first
        nc.vector.memset(hs[0:FB, 0:HS0], 0.0)
        nc.vector.memset(hj[0:FB, 0:JCHUNK], 0.0)
        nc.vector.memset(hj[0:FB, JCHUNK:], 0.0)
        nc.vector.memset(hs[0:FB, HS0:], 0.0)
        nc.vector.memset(t1[0:FB, :], 0.0)
        nc.vector.memset(t2[0:FB, :], 0.0)
        nc.gpsimd.memset(t0[0:FB, :], 0.0)

        # short warm-up during the DMA wait; the K=128 production matmuls
        # sustain the activity window themselves once they start
        wu_ps = psum.tile([128, JCHUNK], F32, tag="g1")  # scratch
        for _ in range(3):
            nc.tensor.matmul(wu_ps[:, 0:512], wu_l[:], wu_r[:])

        # single-writer accumulator stripes; final math happens on host.
        # Two tiles so most of the output DMA is issued mid-kernel.
        HC = ITILES * NJC // 2
        a_lo = singles.tile([128, HC], F32)
        a_hi = singles.tile([128, HC], F32)

        jtiles = {0: (hj, 0), 1: (hj, JCHUNK),
                  2: (t0, 0), 3: (t0, JCHUNK), 4: (t1, 0), 5: (t1, JCHUNK),
                  6: (t2, 0), 7: (t2, JCHUNK)}

        for it in range(ITILES):
            lhs1 = hs[:, it * 128:(it + 1) * 128]
            lhs2 = hs[:, Q + it * 128:Q + (it + 1) * 128]
            # chunk pairs: 4 G1 matmuls then 4 G2 matmuls per pair (one
            # stationary switch per 4 streams)
            for jp in range(NJC // 2):
                g1s, g2s = [], []
                # very first pair: finish chunk a's g1+g2 before touching
                # chunk b, so the first DVE op starts two matmuls earlier
                chunk_major = (it == 0 and jp == 0)
                for jc in (2 * jp, 2 * jp + 1):
                    g1s.append(psum.tile([128, JCHUNK], F32, tag="g1",
                                         name="g1"))
                    g2s.append(psum.tile([128, JCHUNK], F32, tag="g2",
                                         name="g2"))

                def mm(dst, lhs, jc):
                    src, base = jtiles[jc]
                    for h in range(2):
                        nc.tensor.matmul(
                            dst[:, h * 512:(h + 1) * 512], lhs,
                            src[:, base + h * 512:base + (h + 1) * 512])

                if chunk_major:
                    for k, jc in enumerate((2 * jp, 2 * jp + 1)):
                        mm(g1s[k], lhs1, jc)
                        mm(g2s[k], lhs2, jc)
                else:
                    for k, jc in enumerate((2 * jp, 2 * jp + 1)):
                        mm(g1s[k], lhs1, jc)
                    for k, jc in enumerate((2 * jp, 2 * jp + 1)):
                        mm(g2s[k], lhs2, jc)
                for k, jc in enumerate((2 * jp, 2 * jp + 1)):
                    gi = it * NJC + jc
                    dist = work.tile([128, JCHUNK], FP16, tag="dist")
                    nc.scalar.activation(dist[:], g1s[k][:], ACTF.Sqrt)
                    junk = work.tile([128, JCHUNK], mybir.dt.float8e4,
                                     tag="jk")
                    a_t = a_lo if gi < HC else a_hi
                    nc.vector._custom_dve(
                        masked_sdc, out=junk[:], in0=dist[:], in1=g2s[k][:],
                        s1=ENC_C, accum_out=a_t[:, gi % HC:gi % HC + 1])
                if it == ITILES - 1 and jp == 1:
                    # accum stripes for gi 48..59 are complete here
                    nc.sync.dma_start(oa_d[:, HC + 16:HC + 28],
                                      a_hi[:, 16:28])
            if it == ITILES // 2 - 1:
                nc.sync.dma_start(oa_d[:, 0:HC], a_lo[:])
            if it == ITILES - 3:
                nc.sync.dma_start(oa_d[:, HC:HC + 16], a_hi[:, 0:16])

        nc.sync.dma_start(oa_d[:, HC + 28:], a_hi[:, 28:])

    nc.compile()
    return nc


_CACHED_NC = None


def _get_nc():
    global _CACHED_NC
    if _CACHED_NC is None:
        _CACHED_NC = _build_graph()
    return _CACHED_NC


def _prep_inputs(past_ped_positions, ped_positions, indexes, all_radii):
    pos = np.asarray(ped_positions, np.float64)
    past = np.asarray(past_ped_positions, np.float64)
    v = pos - past
    vn = np.hypot(v[:, 0], v[:, 1])
    safe = np.where(vn > 0, vn, 1.0)
    ux = np.where(vn > 0, v[:, 0] / safe, 1.0)
    uy = np.where(vn > 0, v[:, 1] / safe, 0.0)

    px, py = pos[:, 0], pos[:, 1]
    nsq = px * px + py * py
    px_h, px_l = _split(px)
    py_h, py_l = _split(py)
    nsq_h, nsq_l = _split(nsq)
    ones = np.ones(N)
    jf1 = np.stack([px_h, px_l, px_h, py_h, py_l, py_h, ones, ones,
                    nsq_h, nsq_l]).astype(_F16)
    jf2 = jf1[0:8].copy()
    jf = np.concatenate([jf1, jf2,
                         np.zeros((KFP - KF, N), _F16)], axis=0)  # [32, N]

    a = ux / COS_HALF
    b = uy / COS_HALF
    w = (ux * px + uy * py) / COS_HALF
    a_h, a_l = _split(a)
    b_h, b_l = _split(b)
    w_h, w_l = _split(w)
    nq_h, nq_l = _split(nsq + EPS)
    qf1_full = np.stack([-2 * px_h, -2 * px_h, -2 * px_l,
                         -2 * py_h, -2 * py_h, -2 * py_l,
                         nq_h, nq_l, ones, ones])  # [10, N]
    qf2_full = np.stack([a_h, a_h, a_l, b_h, b_h, b_l, -w_h, -w_l])  # [8, N]

    # column c of per-core qf holds local query (c % 128) * ITILES + c // 128
    cidx = np.arange(Q)
    perm = (cidx % 128) * ITILES + cidx // 128

    jw = 2 * JCHUNK
    t0 = np.ascontiguousarray(jf[:, jw:jw + 2 * JCHUNK])
    t1 = np.ascontiguousarray(jf[:, jw + 2 * JCHUNK:jw + 4 * JCHUNK])
    t2 = np.ascontiguousarray(jf[:, jw + 4 * JCHUNK:])
    in_maps = []
    for k in range(NCORES):
        sl = slice(k * Q, (k + 1) * Q)
        # stationaries with the other block's rows zeroed: G1 rows 0-9,
        # G2 rows 10-17 (matmuls are unmasked K=18)
        s1 = np.zeros((KFP, Q), np.float64)
        s2 = np.zeros((KFP, Q), np.float64)
        s1[0:10] = qf1_full[:, sl][:, perm]
        s2[10:18] = qf2_full[:, sl][:, perm]
        hs = np.concatenate([s1.astype(_F16), s2.astype(_F16)], axis=1)
        hj = np.ascontiguousarray(jf[:, :jw])
        in_maps.append({"hs0": np.ascontiguousarray(hs[:, :1280]),
                        "hsr": np.ascontiguousarray(hs[:, 1280:]),
                        "hj": hj, "t0": t0, "t1": t1, "t2": t2})
    return in_maps


def _host_epilogue(res_core, idxf_core, radii_core):
    """[128, 64] encoded accumulator stripes -> [1024] final radii for one
    core. idxf_core/radii_core are [128, ITILES] (local query
    q = p*ITILES + it). Each accumulator lane holds ENC_C*cnt + s for one
    (query, j-chunk); s < ENC_C so floor-divide separates them (+64 absorbs
    downward fp accumulation error in near-empty chunks)."""
    A = np.asarray(res_core["out_a"], np.float64).reshape(128, ITILES, NJC)
    cnt_c = np.floor((A + 64.0) / ENC_C)
    s_c = A - ENC_C * cnt_c
    c = cnt_c.sum(2)
    s = s_c.sum(2)
    mean = (s / np.maximum(c, 1.0)).astype(np.float32)
    r = np.clip(mean * np.float32(SLOPE) + np.float32(OFFS), MIN_R, MAX_R)
    fin = radii_core + idxf_core * (r - radii_core)
    return fin.astype(np.float32).reshape(Q)


def kernel(past_ped_positions, ped_positions, indexes, all_radii,
           _trace=False, _trace_kwargs=None):
    nc = _get_nc()
    in_maps = _prep_inputs(past_ped_positions, ped_positions, indexes,
                           all_radii)
    kw = {}
    if _trace:
        kw = {"trace": True}
        if _trace_kwargs:
            kw.update(_trace_kwargs)
    res = run_bass_kernel_spmd(nc, in_maps, list(range(NCORES)), **kw)
    idxf = np.asarray(indexes).astype(np.float32)
    radii = np.asarray(all_radii, np.float32)
    outs = []
    for k in range(NCORES):
        sl = slice(k * Q, (k + 1) * Q)
        outs.append(_host_epilogue(res.results[k],
                                   idxf[sl].reshape(128, ITILES),
                                   radii[sl].reshape(128, ITILES)))
    out = np.concatenate(outs)
    if _trace:
        kernel.last_results = res
    return out


# revision 26
# speedup vs baseline: 1.0340x; 1.0340x over previous
"""Trainium2 Bass kernel for ArcShapeRadiusConfigVisibleNeighDist.

For each pedestrian i (N=8192):
  heading u_i = normalize(pos_i - past_i)
  over all j: dist_ij = |pos_j - pos_i|, visible iff angle(pos_j-pos_i, u_i)
  in [-35deg, 35deg) and j != i. Output = affine(clip(mean visible dist)).

Key reformulation (no atan2 anywhere):
  visible  <=>  rel . u_i > cos(35deg) * dist  <=>  dot/c > dist
  sq and dot/c are K-small matmuls on the TensorEngine with fp16 hi/lo
  split features (K is free on the PE). Both feature sets are stacked
  into one K=18 block (rows 0-9: G1/sq, rows 10-17: G2/dot); the G1/G2
  stationaries zero the other block's rows, so the matmuls are plain
  unmasked K=18 matmuls.

Clock-gate handling (measured): the PE HAM activity monitor does not
count small-K matmuls as "busy", so sustained K=18 work re-throttles
the array to 1.2 GHz ~3.4us after the last full-array matmul (600ns
per 512-col matmul instead of ~375ns warm). Fix: zero-pad K to 128 so
every production matmul is full-array. Real features sit at partitions
96-113 (the 32-row host tensors land at 96-127 with host-zero rows
114-127); partitions 0-95 are zeroed once on device (base-0 memsets -
non-zero-base engine APs are limited to 32 partitions). The stationary
zero rows select the G1/G2 block and null the pad rows, so K=128
matmuls compute exactly the K=18 product while holding 2.4 GHz. A ~4us
burst of warm-up matmuls on scratch PSUM covers the initial ramp
during the input-DMA wait.

Per 128-query x 1024-j chunk (single fused vector pass per element):
  PE:  g1 = sq (+eps) [128,1024], g2 = dot/c [128,1024]
  ACT: dist = sqrt(g1) -> fp16 [128,1024]
  DVE: custom MASKED_SDC: b = select(g2 > dist, dist + ENC_C, 0),
       accum -> A = ENC_C*cnt + s  (one accumulator carries BOTH the
       visible count and the visible-distance sum; per-chunk s < 2^17
       so the host separates them with a floor-divide).
Host epilogue: cnt = floor((A+64)/ENC_C); s = A - ENC_C*cnt per chunk,
  summed over chunks; r = clip(s/max(cnt,1) * k + b, 0.5, 4.0);
  select by indexes.

Sharding: core k owns queries [k*1024, (k+1)*1024), full j set.
"""

import numpy as np

import concourse.bass as bass
import concourse.bacc as bacc
import concourse.mybir as mybir
import concourse.tile as tile
from contextlib import ExitStack
from concourse.bass_utils import run_bass_kernel_spmd
from concourse.dve_uop import DveOpSpec
import concourse.dve_ops as dvo
from concourse.dve_ops import Spec, Src0, Src1, Zero, C1, select, lower, has_src1
from concourse.dve_ops import AluOp as SAluOp

N = 8192
NCORES = 8
Q = N // NCORES            # 1024 queries per core
ITILES = Q // 128          # 8 partition tiles of queries
JCHUNK = 1024
NJC = N // JCHUNK          # 8 j-chunks per i-tile
EPS = 0.005                # sq guard: keeps diag excluded, sqrt input > 0
COS_HALF = float(np.cos(70.0 * np.pi / 180.0 / 2.0))
MIN_R, MAX_R = 0.5, 4.0
MIN_D, MAX_D = 0.2, 5.0
SLOPE = (MAX_R - MIN_R) / (MAX_D - MIN_D)
OFFS = MIN_R - MIN_D * SLOPE
ENC_C = 131072.0           # 2^17: per-chunk s < 1024*dmax ~ 98e3 < 2^17
KF = 18                    # stacked feature rows: 0-9 G1 (sq), 10-17 G2 (dot)
KFP = 32                   # host pads features to 32 rows (engine ops need
                           # 32-aligned partition bases; rows 18-31 are zero)

F32 = mybir.dt.float32
FP16 = mybir.dt.float16
ACTF = mybir.ActivationFunctionType
_F16 = np.float16


def register_masked_sdc():
    """Runtime-register the fused DVE op:
    out = select(in0 > in1, in1 + s1, 0), accum_out = sum(out).
    With s1 = ENC_C the accumulator encodes ENC_C*count + sum(dist) in one
    fp32 lane. The per-NEFF uop table is generated from OPS, so appending
    at runtime is sufficient (no firmware change)."""
    name = "MASKED_SDC_ANT"
    if name in dvo._SUB_OPCODE_FOR_NAME:
        return getattr(dvo, name)

    def _ref(in0, in1, s0, s1, imm2):
        b = np.where(in1.astype(np.float32) > in0,
                     in0.astype(np.float32) + np.float32(s1),
                     0.0).astype(np.float32)
        return b, b.reshape(b.shape[0], -1).sum(axis=-1, keepdims=True)

    # dist rides in0 (SBUF fp16), the PSUM fp32 operand rides in1
    spec = Spec(body=select(Src1 > Src0, Src0 + C1, Zero), accum=SAluOp.ADD,
                reference=_ref)
    row = max(dvo._SUB_OPCODE_FOR_NAME.values()) + 1
    assert row < 0x20
    dvo._SUB_OPCODE_FOR_NAME[name] = row
    op = dvo.DveOp(name, spec, subdim=False, uops_sha={})
    for ver in ("v3", "v4"):
        s = DveOpSpec(name=name, opcode=row, uops=lower(spec, ver=ver),
                      rd1_en=has_src1(spec))
        op.uops_sha[ver] = s.sha(ver)
    dvo.OPS.append(op)
    dvo.CUSTOM_DVE_SPECS[name] = spec
    setattr(dvo, name, op)
    return op


def _split(x):
    """Split f64 array into fp16 hi + fp16 lo (as f64 of exact fp16 values)."""
    h = x.astype(_F16).astype(np.float64)
    l = (x - h).astype(_F16).astype(np.float64)
    return h, l


def _build_graph():
    masked_sdc = register_masked_sdc()
    nc = bacc.Bacc("TRN2", target_bir_lowering=False, debug=False,
                   num_devices=NCORES)
    # One [32, *] feature stack (rows 0-9 G1, 10-17 G2, 18-31 host zeros);
    # G1/G2 separation happens via zero rows in the two stationary blocks.
    # hs carries both stationaries, hj the first two j-chunks.
    jw = 2 * JCHUNK
    hs_d = nc.dram_tensor("hs", [KFP, 2 * Q], FP16, kind="ExternalInput")
    hj_d = nc.dram_tensor("hj", [KFP, jw], FP16, kind="ExternalInput")
    t0_d = nc.dram_tensor("t0", [KFP, 2 * JCHUNK], FP16, kind="ExternalInput")
    t1_d = nc.dram_tensor("t1", [KFP, 2 * JCHUNK], FP16, kind="ExternalInput")
    t2_d = nc.dram_tensor("t2", [KFP, 2 * JCHUNK], FP16, kind="ExternalInput")
    oa_d = nc.dram_tensor("out_a", [128, ITILES * NJC], F32,
                          kind="ExternalOutput")

    with tile.TileContext(nc) as tc, ExitStack() as ctx:
        singles = ctx.enter_context(tc.tile_pool(name="singles", bufs=1))
        psum = ctx.enter_context(tc.tile_pool(name="psum", bufs=2, space="PSUM"))
        work = ctx.enter_context(tc.tile_pool(name="work", bufs=4))

        # warm-up operands: zeroed via the (idle) Vector queue so the
        # initial warm-up matmuls can start during the input-DMA wait
        wu_l = singles.tile([128, 128], FP16)
        wu_r = singles.tile([128, 512], FP16)
        nc.gpsimd.memset(wu_l[:], 0.0)
        nc.gpsimd.memset(wu_r[:], 0.0)

        # All matmul operands are [128, *]: rows 0-17 carry the real
        # features (DMA), rows 18-127 are zeroed on device. Full-array
        # K=128 matmuls keep the PE activity monitor happy so the array
        # holds its 2.4 GHz clock (small-K matmuls don't count as "busy"
        # and the PE re-throttles to 1.2 GHz ~3.4us after the last
        # full-array matmul - measured as 600ns vs 215ns per matmul).
        hs = singles.tile([128, 2 * Q], FP16)
        hj = singles.tile([128, jw], FP16)
        t0 = singles.tile([128, 2 * JCHUNK], FP16)
        t1 = singles.tile([128, 2 * JCHUNK], FP16)
        t2 = singles.tile([128, 2 * JCHUNK], FP16)
        # real features live at partitions 96-113 (zeros 114-127 come from
        # the host pad); partitions 0-95 are zeroed on device. Engine APs
        # with non-zero partition base are limited to 32 partitions, so the
        # big memsets must be the base-0 ones. Queue placement orders the
        # chunk-0 dependencies (hs, hj) first on every queue.
        FB = 128 - KFP  # feature base partition (96)
        nc.sync.dma_start(hs[FB:128, :], hs_d[:])
        nc.gpsimd.dma_start(hj[FB:128, :], hj_d[:])
        nc.vector.memset(hs[0:FB, :], 0.0)               # stationaries pad
        nc.gpsimd.memset(hj[0:FB, :], 0.0)               # head j-chunks pad
        nc.gpsimd.dma_start(t0[FB:128, :], t0_d[:])
        nc.sync.dma_start(t1[FB:128, :], t1_d[:])
        nc.gpsimd.dma_start(t2[FB:128, :], t2_d[:])
        nc.gpsimd.memset(t0[0:FB, :], 0.0)
        nc.vector.memset(t1[0:FB, :], 0.0)
        nc.vector.memset(t2[0:FB, :], 0.0)

        # initial warm-up: ~4us of full-array matmuls during the DMA wait;
        # the K=128 production matmuls then keep the array warm themselves
        wu_ps = psum.tile([128, JCHUNK], F32, tag="g1")  # scratch
        for _ in range(6):
            nc.tensor.matmul(wu_ps[:, 0:512], wu_l[:], wu_r[:])

        # single-writer accumulator stripes; final math happens on host.
        # Two tiles so most of the output DMA is issued mid-kernel.
        HC = ITILES * NJC // 2
        a_lo = singles.tile([128, HC], F32)
        a_hi = singles.tile([128, HC], F32)

        jtiles = {0: (hj, 0), 1: (hj, JCHUNK),
                  2: (t0, 0), 3: (t0, JCHUNK), 4: (t1, 0), 5: (t1, JCHUNK),
                  6: (t2, 0), 7: (t2, JCHUNK)}

        for it in range(ITILES):
            lhs1 = hs[:, it * 128:(it + 1) * 128]
            lhs2 = hs[:, Q + it * 128:Q + (it + 1) * 128]
            # chunk pairs: 4 G1 matmuls then 4 G2 matmuls per pair (one
            # stationary switch per 4 streams)
            for jp in range(NJC // 2):
                g1s, g2s = [], []
                # very first pair: finish chunk a's g1+g2 before touching
                # chunk b, so the first DVE op starts two matmuls earlier
                chunk_major = (it == 0 and jp == 0)
                for jc in (2 * jp, 2 * jp + 1):
                    g1s.append(psum.tile([128, JCHUNK], F32, tag="g1",
                                         name="g1"))
                    g2s.append(psum.tile([128, JCHUNK], F32, tag="g2",
                                         name="g2"))

                def mm(dst, lhs, jc):
                    src, base = jtiles[jc]
                    for h in range(2):
                        nc.tensor.matmul(
                            dst[:, h * 512:(h + 1) * 512], lhs,
                            src[:, base + h * 512:base + (h + 1) * 512])

                if chunk_major:
                    for k, jc in enumerate((2 * jp, 2 * jp + 1)):
                        mm(g1s[k], lhs1, jc)
                        mm(g2s[k], lhs2, jc)
                else:
                    for k, jc in enumerate((2 * jp, 2 * jp + 1)):
                        mm(g1s[k], lhs1, jc)
                    for k, jc in enumerate((2 * jp, 2 * jp + 1)):
                        mm(g2s[k], lhs2, jc)
                for k, jc in enumerate((2 * jp, 2 * jp + 1)):
                    gi = it * NJC + jc
                    dist = work.tile([128, JCHUNK], FP16, tag="dist")
                    nc.scalar.activation(dist[:], g1s[k][:], ACTF.Sqrt)
                    junk = work.tile([128, JCHUNK], mybir.dt.float8e4,
                                     tag="jk")
                    a_t = a_lo if gi < HC else a_hi
                    nc.vector._custom_dve(
                        masked_sdc, out=junk[:], in0=dist[:], in1=g2s[k][:],
                        s1=ENC_C, accum_out=a_t[:, gi % HC:gi % HC + 1])
                if it == ITILES - 1 and jp == 1:
                    # accum stripes for gi 48..59 are complete here
                    nc.sync.dma_start(oa_d[:, HC + 16:HC + 28],
                                      a_hi[:, 16:28])
            if it == ITILES // 2 - 1:
                nc.sync.dma_start(oa_d[:, 0:HC], a_lo[:])
            if it == ITILES - 3:
                nc.sync.dma_start(oa_d[:, HC:HC + 16], a_hi[:, 0:16])

        nc.sync.dma_start(oa_d[:, HC + 28:], a_hi[:, 28:])

    nc.compile()
    return nc


_CACHED_NC = None


def _get_nc():
    global _CACHED_NC
    if _CACHED_NC is None:
        _CACHED_NC = _build_graph()
    return _CACHED_NC


def _prep_inputs(past_ped_positions, ped_positions, indexes, all_radii):
    pos = np.asarray(ped_positions, np.float64)
    past = np.asarray(past_ped_positions, np.float64)
    v = pos - past
    vn = np.hypot(v[:, 0], v[:, 1])
    safe = np.where(vn > 0, vn, 1.0)
    ux = np.where(vn > 0, v[:, 0] / safe, 1.0)
    uy = np.where(vn > 0, v[:, 1] / safe, 0.0)

    px, py = pos[:, 0], pos[:, 1]
    nsq = px * px + py * py
    px_h, px_l = _split(px)
    py_h, py_l = _split(py)
    nsq_h, nsq_l = _split(nsq)
    ones = np.ones(N)
    jf1 = np.stack([px_h, px_l, px_h, py_h, py_l, py_h, ones, ones,
                    nsq_h, nsq_l]).astype(_F16)
    jf2 = jf1[0:8].copy()
    jf = np.concatenate([jf1, jf2,
                         np.zeros((KFP - KF, N), _F16)], axis=0)  # [32, N]

    a = ux / COS_HALF
    b = uy / COS_HALF
    w = (ux * px + uy * py) / COS_HALF
    a_h, a_l = _split(a)
    b_h, b_l = _split(b)
    w_h, w_l = _split(w)
    nq_h, nq_l = _split(nsq + EPS)
    qf1_full = np.stack([-2 * px_h, -2 * px_h, -2 * px_l,
                         -2 * py_h, -2 * py_h, -2 * py_l,
                         nq_h, nq_l, ones, ones])  # [10, N]
    qf2_full = np.stack([a_h, a_h, a_l, b_h, b_h, b_l, -w_h, -w_l])  # [8, N]

    # column c of per-core qf holds local query (c % 128) * ITILES + c // 128
    cidx = np.arange(Q)
    perm = (cidx % 128) * ITILES + cidx // 128

    jw = 2 * JCHUNK
    t0 = np.ascontiguousarray(jf[:, jw:jw + 2 * JCHUNK])
    t1 = np.ascontiguousarray(jf[:, jw + 2 * JCHUNK:jw + 4 * JCHUNK])
    t2 = np.ascontiguousarray(jf[:, jw + 4 * JCHUNK:])
    in_maps = []
    for k in range(NCORES):
        sl = slice(k * Q, (k + 1) * Q)
        # stationaries with the other block's rows zeroed: G1 rows 0-9,
        # G2 rows 10-17 (matmuls are unmasked K=18)
        s1 = np.zeros((KFP, Q), np.float64)
        s2 = np.zeros((KFP, Q), np.float64)
        s1[0:10] = qf1_full[:, sl][:, perm]
        s2[10:18] = qf2_full[:, sl][:, perm]
        hs = np.concatenate([s1.astype(_F16), s2.astype(_F16)], axis=1)
        hj = np.ascontiguousarray(jf[:, :jw])
        in_maps.append({"hs": hs, "hj": hj, "t0": t0, "t1": t1, "t2": t2})
    return in_maps


def _host_epilogue(res_core, idxf_core, radii_core):
    """[128, 64] encoded accumulator stripes -> [1024] final radii for one
    core. idxf_core/radii_core are [128, ITILES] (local query
    q = p*ITILES + it). Each accumulator lane holds ENC_C*cnt + s for one
    (query, j-chunk); s < ENC_C so floor-divide separates them (+64 absorbs
    downward fp accumulation error in near-empty chunks)."""
    A = np.asarray(res_core["out_a"], np.float64).reshape(128, ITILES, NJC)
    cnt_c = np.floor((A + 64.0) / ENC_C)
    s_c = A - ENC_C * cnt_c
    c = cnt_c.sum(2)
    s = s_c.sum(2)
    mean = (s / np.maximum(c, 1.0)).astype(np.float32)
    r = np.clip(mean * np.float32(SLOPE) + np.float32(OFFS), MIN_R, MAX_R)
    fin = radii_core + idxf_core * (r - radii_core)
    return fin.astype(np.float32).reshape(Q)


def kernel(past_ped_positions, ped_positions, indexes, all_radii,
           _trace=False, _trace_kwargs=None):
    nc = _get_nc()
    in_maps = _prep_inputs(past_ped_positions, ped_positions, indexes,
                           all_radii)
    kw = {}
    if _trace:
        kw = {"trace": True}
        if _trace_kwargs:
            kw.update(_trace_kwargs)
    res = run_bass_kernel_spmd(nc, in_maps, list(range(NCORES)), **kw)
    idxf = np.asarray(indexes).astype(np.float32)
    radii = np.asarray(all_radii, np.float32)
    outs = []
    for k in range(NCORES):
        sl = slice(k * Q, (k + 1) * Q)
        outs.append(_host_epilogue(res.results[k],
                                   idxf[sl].reshape(128, ITILES),
                                   radii[sl].reshape(128, ITILES)))
    out = np.concatenate(outs)
    if _trace:
        kernel.last_results = res
    return out
